# revision 45
# baseline (speedup 1.0000x reference)
"""Trainium2 Bass kernel for an 8-expert top-2 MoE block (T=2048, D=1024, H=4096).

Strategy (expert-parallel, sparse dispatch):
  - Host computes the (tiny) gate: router logits, top-2 selection, softmax
    combine weights.  This *is* the sharding decision: tokens are dispatched
    (gathered) to the core that owns their expert.
  - Core e holds expert e's W1/W2 and runs the FFN only on the tokens routed
    to it (padded to a common capacity C so all 8 cores run one SPMD program).
  - On device: H^T = relu(W1^T X^T + b1), Y^T = (W2^T H^T + b2) * comb, all
    in transposed-activation layout so the contraction dim is always on
    SBUF partitions.
  - Host scatter-adds the per-expert outputs back to the full (T, D) output.
"""

import os
import sys

for p in ("/opt/trn_rl_repo",):
    if p not in sys.path and os.path.isdir(p):
        sys.path.insert(0, p)

# The kernel needs the axon-tunneled NeuronCores; don't let a stray
# JAX_PLATFORMS=cpu (set by some harnesses for the reference) hide them.
if "jax" not in sys.modules and os.environ.get("JAX_PLATFORMS", "") == "cpu":
    del os.environ["JAX_PLATFORMS"]

from contextlib import ExitStack

import numpy as np

import concourse.bass as bass
import concourse.bacc as bacc
import concourse.mybir as mybir
import concourse.tile as tile
from concourse.bass_utils import run_bass_kernel_spmd

T, D, H, E = 2048, 1024, 4096, 8
DC, HC = D // 128, H // 128  # 8, 32 chunks of 128
F32 = mybir.dt.float32
F32R = mybir.dt.float32r  # fp32 bits, single-pass PE mode (4x faster, reduced precision)
BF16 = mybir.dt.bfloat16
FP16 = mybir.dt.float16

# "f32r": fp32 storage, single-pass reduced-precision matmul (~2e-4 rel err)
# "fp16": fp16 operands (~4e-4 rel err), halves weight DMA + faster LDWEIGHTS
# "bf16": bf16 operands (~3e-3 rel err), same speed as fp16
MM_MODE = "fp16"

_prog_cache = {}


def _ntiles(C):
    """Split C (a multiple of 32) into equal-ish chunks of <=512, multiples of 32."""
    nt = -(-C // 512)
    m = C // 32
    sizes = []
    for i in range(nt):
        k = m // nt + (1 if i < m % nt else 0)
        sizes.append(k * 32)
    out, n0 = [], 0
    for s in sizes:
        out.append((n0, s))
        n0 += s
    return out


def _build_program(C, mode):
    """One SPMD program: dense FFN of one expert over C (padded) tokens."""
    MMDT = {"f32r": F32R, "fp16": FP16, "bf16": BF16}[mode]
    nc = bacc.Bacc("TRN2", target_bir_lowering=False, debug=False)

    xg_d = nc.dram_tensor("xg", [128, DC, C], MMDT, kind="ExternalInput")
    w1_d = nc.dram_tensor("w1t", [HC, 128, DC, 128], MMDT, kind="ExternalInput")
    w2_d = nc.dram_tensor("w2t", [DC, 128, HC, 128], MMDT, kind="ExternalInput")
    b1_d = nc.dram_tensor("b1h", [128, HC], F32, kind="ExternalInput")
    b2_d = nc.dram_tensor("b2h", [128, DC], F32, kind="ExternalInput")
    comb_d = nc.dram_tensor("comb", [1, C], F32, kind="ExternalInput")
    out_d = nc.dram_tensor("ygT", [DC, 128, C], F32, kind="ExternalOutput")

    ntiles = _ntiles(C)
    NSZ = ntiles[0][1]

    with tile.TileContext(nc) as tc, ExitStack() as ctx:
        const = ctx.enter_context(tc.tile_pool(name="const", bufs=1))
        w1p = ctx.enter_context(tc.tile_pool(name="w1p", bufs=6))
        w2p = ctx.enter_context(tc.tile_pool(name="w2p", bufs=4))
        hp = ctx.enter_context(tc.tile_pool(name="hp", bufs=1))
        op = ctx.enter_context(tc.tile_pool(name="outp", bufs=4))
        psp = ctx.enter_context(tc.tile_pool(name="ps", bufs=3, space="PSUM"))

        # Head-of-stream: issue w1[0..1] ahead of everything so the first
        # matmul's weights transfer in parallel with the xg chunks.
        w1_head = []
        for hc in range(2):
            w1h = w1p.tile([128, DC, 128], MMDT, name=f"w1h{hc}", tag="w1h")
            nc.sync.dma_start(w1h[:], w1_d[hc])
            w1_head.append(w1h)

        xg = const.tile([128, DC, C], MMDT)
        for dc in range(0, DC, 2):  # split across DMA queues
            nc.sync.dma_start(xg[:, dc : dc + 2, :], xg_d[:, dc : dc + 2, :])
        # small consts on the (idle) gpsimd SWDGE path, off Sync's queue
        b1s = const.tile([128, HC], F32)
        nc.gpsimd.dma_start(b1s[:], b1_d[:])
        b2s = const.tile([128, DC], F32)
        nc.gpsimd.dma_start(b2s[:], b2_d[:])
        combrow = const.tile([1, C], F32)
        nc.gpsimd.dma_start(combrow[:], comb_d[:])
        combb = const.tile([128, C], F32)
        nc.gpsimd.partition_broadcast(combb[:], combrow[:])

        hT = hp.tile([128, HC, C], MMDT)

        # PE warmup: ~5.5us of dummy matmuls during the initial DMA wait so
        # the HAM clock gate reaches 8/8 just as the first real data lands
        # (ends <3.4us before the first real matmul, so no re-throttle).
        warm = const.tile([128, 512], BF16)
        nc.gpsimd.memset(warm[:], 0.0)
        wps = psp.tile([128, NSZ], F32, name="wps", tag="ps0")
        for _ in range(42):
            nc.tensor.matmul(
                wps[:, :256], warm[:, :128], warm[:, :256], start=True, stop=True
            )

        # Phase 1: H^T[h, t] = relu(sum_d W1[d, h] * X^T[d, t] + b1[h])
        for hc in range(HC):
            if hc < 2:
                w1h = w1_head[hc]
            else:
                w1h = w1p.tile([128, DC, 128], MMDT, name=f"w1h{hc}", tag="w1h")
                nc.sync.dma_start(w1h[:], w1_d[hc])
            pss = [psp.tile([128, NSZ], F32, name=f"ps{i}", tag=f"ps{i}") for i in range(len(ntiles))]
            for dc in range(DC):
                for ps, (n0, nsz) in zip(pss, ntiles):
                    nc.tensor.matmul(
                        ps[:, :nsz],
                        w1h[:, dc, :],
                        xg[:, dc, n0 : n0 + nsz],
                        start=(dc == 0),
                        stop=(dc == DC - 1),
                    )
            for ps, (n0, nsz) in zip(pss, ntiles):
                nc.scalar.activation(
                    hT[:, hc, n0 : n0 + nsz],
                    ps[:, :nsz],
                    mybir.ActivationFunctionType.Relu,
                    bias=b1s[:, hc : hc + 1],
                )

        # Phase 2: Y^T[d, t] = (sum_h W2[h, d] * H^T[h, t] + b2[d]) * comb[t]
        for dc in range(DC):
            w2d = w2p.tile([128, HC, 128], MMDT)
            for q in range(4):  # split across DMA queues
                nc.sync.dma_start(
                    w2d[:, q * 8 : (q + 1) * 8, :], w2_d[dc, :, q * 8 : (q + 1) * 8, :]
                )
            for i, (n0, nsz) in enumerate(ntiles):
                ps = psp.tile([128, NSZ], F32, name=f"ps{i}", tag=f"ps{i}")
                for hc in range(HC):
                    nc.tensor.matmul(
                        ps[:, :nsz],
                        w2d[:, hc, :],
                        hT[:, hc, n0 : n0 + nsz],
                        start=(hc == 0),
                        stop=(hc == HC - 1),
                    )
                ot = op.tile([128, NSZ], F32)
                nc.vector.scalar_tensor_tensor(
                    ot[:, :nsz],
                    ps[:, :nsz],
                    b2s[:, dc : dc + 1],
                    combb[:, n0 : n0 + nsz],
                    op0=mybir.AluOpType.add,
                    op1=mybir.AluOpType.mult,
                )
                nc.sync.dma_start(out_d[dc, :, n0 : n0 + nsz], ot[:, :nsz])

    nc.compile()
    return nc


def _route(xs, Wg, k):
    """Top-k routing + softmax combine weights, mirroring jax.lax.top_k
    (descending, ties broken by lower index) + softmax over the k logits."""
    router = xs @ Wg.T  # (T, E) fp32
    t = np.arange(xs.shape[0])[:, None]
    sel = np.zeros((xs.shape[0], k), np.int64)
    masked = router.copy()
    for j in range(k):
        sel[:, j] = np.argmax(masked, axis=1)
        masked[t[:, 0], sel[:, j]] = -np.inf
    logits = router[t, sel]  # (T, k), descending
    ex = np.exp((logits - logits[:, :1]).astype(np.float32))
    wgt = (ex / ex.sum(axis=1, keepdims=True)).astype(np.float32)
    return sel, wgt


def _prep_core_inputs(xs, W1e, b1e, W2e, b2e, idx, wgt, C):
    import ml_dtypes

    mmdt = {"f32r": np.float32, "fp16": np.float16, "bf16": ml_dtypes.bfloat16}[
        MM_MODE
    ]
    n = idx.shape[0]
    xsg = np.zeros((C, D), np.float32)
    xsg[:n] = xs[idx]
    xg = np.ascontiguousarray(xsg.T.reshape(DC, 128, C).transpose(1, 0, 2)).astype(mmdt)
    w1t = np.ascontiguousarray(W1e.reshape(DC, 128, HC, 128).transpose(2, 1, 0, 3)).astype(mmdt)
    w2t = np.ascontiguousarray(W2e.reshape(HC, 128, DC, 128).transpose(2, 1, 0, 3)).astype(mmdt)
    b1h = np.ascontiguousarray(b1e.reshape(HC, 128).T)
    b2h = np.ascontiguousarray(b2e.reshape(DC, 128).T)
    comb = np.zeros((1, C), np.float32)
    comb[0, :n] = wgt
    return {"xg": xg, "w1t": w1t, "w2t": w2t, "b1h": b1h, "b2h": b2h, "comb": comb}


def _run(inputs, trace=False, **rk):
    xs = np.asarray(inputs["xs"], np.float32)
    top_k = int(inputs["top_k"])
    Wg = np.asarray(inputs["Wg"], np.float32)
    W1 = np.asarray(inputs["W1"], np.float32)
    b1 = np.asarray(inputs["b1"], np.float32)
    W2 = np.asarray(inputs["W2"], np.float32)
    b2 = np.asarray(inputs["b2"], np.float32)

    sel2, wgt2 = _route(xs, Wg, top_k)
    sel = sel2.ravel()
    wgt = wgt2.ravel()
    tok = np.repeat(np.arange(T), top_k)
    idxs, wgts = [], []
    for e in range(E):
        m = sel == e
        idxs.append(tok[m])
        wgts.append(wgt[m].astype(np.float32))
    C = max(128, -(-max(len(ix) for ix in idxs) // 32) * 32)

    key = (C, MM_MODE)
    if key not in _prog_cache:
        _prog_cache[key] = _build_program(C, MM_MODE)
    nc = _prog_cache[key]

    in_maps = [
        _prep_core_inputs(xs, W1[e], b1[e], W2[e], b2[e], idxs[e], wgts[e], C)
        for e in range(E)
    ]
    res = run_bass_kernel_spmd(nc, in_maps, core_ids=list(range(E)), trace=trace, **rk)

    out = np.zeros((T, D), np.float32)
    for e in range(E):
        n = len(idxs[e])
        ygT = res.results[e]["ygT"].reshape(D, C)
        out[idxs[e]] += ygT[:, :n].T
    return out, res


def kernel(**inputs) -> np.ndarray:
    out, _ = _run(inputs)
    return out


# revision 46
# speedup vs baseline: 1.0030x; 1.0030x over previous
"""Trainium2 Bass kernel for an 8-expert top-2 MoE block (T=2048, D=1024, H=4096).

Strategy (expert-parallel, sparse dispatch):
  - Host computes the (tiny) gate: router logits, top-2 selection, softmax
    combine weights.  This *is* the sharding decision: tokens are dispatched
    (gathered) to the core that owns their expert.
  - Core e holds expert e's W1/W2 and runs the FFN only on the tokens routed
    to it (padded to a common capacity C so all 8 cores run one SPMD program).
  - On device: H^T = relu(W1^T X^T + b1), Y^T = (W2^T H^T + b2) * comb, all
    in transposed-activation layout so the contraction dim is always on
    SBUF partitions.
  - Host scatter-adds the per-expert outputs back to the full (T, D) output.
"""

import os
import sys

for p in ("/opt/trn_rl_repo",):
    if p not in sys.path and os.path.isdir(p):
        sys.path.insert(0, p)

# The kernel needs the axon-tunneled NeuronCores; don't let a stray
# JAX_PLATFORMS=cpu (set by some harnesses for the reference) hide them.
if "jax" not in sys.modules and os.environ.get("JAX_PLATFORMS", "") == "cpu":
    del os.environ["JAX_PLATFORMS"]

from contextlib import ExitStack

import numpy as np

import concourse.bass as bass
import concourse.bacc as bacc
import concourse.mybir as mybir
import concourse.tile as tile
from concourse.bass_utils import run_bass_kernel_spmd

T, D, H, E = 2048, 1024, 4096, 8
DC, HC = D // 128, H // 128  # 8, 32 chunks of 128
F32 = mybir.dt.float32
F32R = mybir.dt.float32r  # fp32 bits, single-pass PE mode (4x faster, reduced precision)
BF16 = mybir.dt.bfloat16
FP16 = mybir.dt.float16

# "f32r": fp32 storage, single-pass reduced-precision matmul (~2e-4 rel err)
# "fp16": fp16 operands (~4e-4 rel err), halves weight DMA + faster LDWEIGHTS
# "bf16": bf16 operands (~3e-3 rel err), same speed as fp16
MM_MODE = "fp16"

_prog_cache = {}


def _ntiles(C):
    """Split C (a multiple of 32) into equal-ish chunks of <=512, multiples of 32."""
    nt = -(-C // 512)
    m = C // 32
    sizes = []
    for i in range(nt):
        k = m // nt + (1 if i < m % nt else 0)
        sizes.append(k * 32)
    out, n0 = [], 0
    for s in sizes:
        out.append((n0, s))
        n0 += s
    return out


def _build_program(C, mode):
    """One SPMD program: dense FFN of one expert over C (padded) tokens."""
    MMDT = {"f32r": F32R, "fp16": FP16, "bf16": BF16}[mode]
    nc = bacc.Bacc("TRN2", target_bir_lowering=False, debug=False)

    xg_d = nc.dram_tensor("xg", [128, DC, C], MMDT, kind="ExternalInput")
    w1_d = nc.dram_tensor("w1t", [HC, 128, DC, 128], MMDT, kind="ExternalInput")
    w2_d = nc.dram_tensor("w2t", [DC, 128, HC, 128], MMDT, kind="ExternalInput")
    b1_d = nc.dram_tensor("b1h", [128, HC], F32, kind="ExternalInput")
    b2_d = nc.dram_tensor("b2h", [128, DC], F32, kind="ExternalInput")
    comb_d = nc.dram_tensor("comb", [1, C], F32, kind="ExternalInput")
    out_d = nc.dram_tensor("ygT", [DC, 128, C], MMDT, kind="ExternalOutput")

    ntiles = _ntiles(C)
    NSZ = ntiles[0][1]

    with tile.TileContext(nc) as tc, ExitStack() as ctx:
        const = ctx.enter_context(tc.tile_pool(name="const", bufs=1))
        w1p = ctx.enter_context(tc.tile_pool(name="w1p", bufs=6))
        w2p = ctx.enter_context(tc.tile_pool(name="w2p", bufs=4))
        hp = ctx.enter_context(tc.tile_pool(name="hp", bufs=1))
        op = ctx.enter_context(tc.tile_pool(name="outp", bufs=4))
        psp = ctx.enter_context(tc.tile_pool(name="ps", bufs=3, space="PSUM"))

        # Head-of-stream: issue w1[0..1] ahead of everything so the first
        # matmul's weights transfer in parallel with the xg chunks.
        w1_head = []
        for hc in range(2):
            w1h = w1p.tile([128, DC, 128], MMDT, name=f"w1h{hc}", tag="w1h")
            nc.sync.dma_start(w1h[:], w1_d[hc])
            w1_head.append(w1h)

        xg = const.tile([128, DC, C], MMDT)
        for dc in range(0, DC, 2):  # split across DMA queues
            nc.sync.dma_start(xg[:, dc : dc + 2, :], xg_d[:, dc : dc + 2, :])
        # small consts on the (idle) gpsimd SWDGE path, off Sync's queue
        b1s = const.tile([128, HC], F32)
        nc.gpsimd.dma_start(b1s[:], b1_d[:])
        b2s = const.tile([128, DC], F32)
        nc.gpsimd.dma_start(b2s[:], b2_d[:])
        combrow = const.tile([1, C], F32)
        nc.gpsimd.dma_start(combrow[:], comb_d[:])
        combb = const.tile([128, C], F32)
        nc.gpsimd.partition_broadcast(combb[:], combrow[:])

        hT = hp.tile([128, HC, C], MMDT)

        # Phase 1: H^T[h, t] = relu(sum_d W1[d, h] * X^T[d, t] + b1[h])
        for hc in range(HC):
            if hc < 2:
                w1h = w1_head[hc]
            else:
                w1h = w1p.tile([128, DC, 128], MMDT, name=f"w1h{hc}", tag="w1h")
                nc.sync.dma_start(w1h[:], w1_d[hc])
            pss = [psp.tile([128, NSZ], F32, name=f"ps{i}", tag=f"ps{i}") for i in range(len(ntiles))]
            for dc in range(DC):
                for ps, (n0, nsz) in zip(pss, ntiles):
                    nc.tensor.matmul(
                        ps[:, :nsz],
                        w1h[:, dc, :],
                        xg[:, dc, n0 : n0 + nsz],
                        start=(dc == 0),
                        stop=(dc == DC - 1),
                    )
            for ps, (n0, nsz) in zip(pss, ntiles):
                nc.scalar.activation(
                    hT[:, hc, n0 : n0 + nsz],
                    ps[:, :nsz],
                    mybir.ActivationFunctionType.Relu,
                    bias=b1s[:, hc : hc + 1],
                )

        # Phase 2: Y^T[d, t] = (sum_h W2[h, d] * H^T[h, t] + b2[d]) * comb[t]
        for dc in range(DC):
            w2d = w2p.tile([128, HC, 128], MMDT)
            for q in range(4):  # split across DMA queues
                nc.sync.dma_start(
                    w2d[:, q * 8 : (q + 1) * 8, :], w2_d[dc, :, q * 8 : (q + 1) * 8, :]
                )
            for i, (n0, nsz) in enumerate(ntiles):
                ps = psp.tile([128, NSZ], F32, name=f"ps{i}", tag=f"ps{i}")
                for hc in range(HC):
                    nc.tensor.matmul(
                        ps[:, :nsz],
                        w2d[:, hc, :],
                        hT[:, hc, n0 : n0 + nsz],
                        start=(hc == 0),
                        stop=(hc == HC - 1),
                    )
                ot = op.tile([128, NSZ], MMDT)
                nc.vector.scalar_tensor_tensor(
                    ot[:, :nsz],
                    ps[:, :nsz],
                    b2s[:, dc : dc + 1],
                    combb[:, n0 : n0 + nsz],
                    op0=mybir.AluOpType.add,
                    op1=mybir.AluOpType.mult,
                )
                nc.sync.dma_start(out_d[dc, :, n0 : n0 + nsz], ot[:, :nsz])

    nc.compile()
    return nc


def _route(xs, Wg, k):
    """Top-k routing + softmax combine weights, mirroring jax.lax.top_k
    (descending, ties broken by lower index) + softmax over the k logits."""
    router = xs @ Wg.T  # (T, E) fp32
    t = np.arange(xs.shape[0])[:, None]
    sel = np.zeros((xs.shape[0], k), np.int64)
    masked = router.copy()
    for j in range(k):
        sel[:, j] = np.argmax(masked, axis=1)
        masked[t[:, 0], sel[:, j]] = -np.inf
    logits = router[t, sel]  # (T, k), descending
    ex = np.exp((logits - logits[:, :1]).astype(np.float32))
    wgt = (ex / ex.sum(axis=1, keepdims=True)).astype(np.float32)
    return sel, wgt


def _prep_core_inputs(xs, W1e, b1e, W2e, b2e, idx, wgt, C):
    import ml_dtypes

    mmdt = {"f32r": np.float32, "fp16": np.float16, "bf16": ml_dtypes.bfloat16}[
        MM_MODE
    ]
    n = idx.shape[0]
    xsg = np.zeros((C, D), np.float32)
    xsg[:n] = xs[idx]
    xg = np.ascontiguousarray(xsg.T.reshape(DC, 128, C).transpose(1, 0, 2)).astype(mmdt)
    w1t = np.ascontiguousarray(W1e.reshape(DC, 128, HC, 128).transpose(2, 1, 0, 3)).astype(mmdt)
    w2t = np.ascontiguousarray(W2e.reshape(HC, 128, DC, 128).transpose(2, 1, 0, 3)).astype(mmdt)
    b1h = np.ascontiguousarray(b1e.reshape(HC, 128).T)
    b2h = np.ascontiguousarray(b2e.reshape(DC, 128).T)
    comb = np.zeros((1, C), np.float32)
    comb[0, :n] = wgt
    return {"xg": xg, "w1t": w1t, "w2t": w2t, "b1h": b1h, "b2h": b2h, "comb": comb}


def _run(inputs, trace=False, **rk):
    xs = np.asarray(inputs["xs"], np.float32)
    top_k = int(inputs["top_k"])
    Wg = np.asarray(inputs["Wg"], np.float32)
    W1 = np.asarray(inputs["W1"], np.float32)
    b1 = np.asarray(inputs["b1"], np.float32)
    W2 = np.asarray(inputs["W2"], np.float32)
    b2 = np.asarray(inputs["b2"], np.float32)

    sel2, wgt2 = _route(xs, Wg, top_k)
    sel = sel2.ravel()
    wgt = wgt2.ravel()
    tok = np.repeat(np.arange(T), top_k)
    idxs, wgts = [], []
    for e in range(E):
        m = sel == e
        idxs.append(tok[m])
        wgts.append(wgt[m].astype(np.float32))
    C = max(128, -(-max(len(ix) for ix in idxs) // 32) * 32)

    key = (C, MM_MODE)
    if key not in _prog_cache:
        _prog_cache[key] = _build_program(C, MM_MODE)
    nc = _prog_cache[key]

    in_maps = [
        _prep_core_inputs(xs, W1[e], b1[e], W2[e], b2[e], idxs[e], wgts[e], C)
        for e in range(E)
    ]
    res = run_bass_kernel_spmd(nc, in_maps, core_ids=list(range(E)), trace=trace, **rk)

    out = np.zeros((T, D), np.float32)
    for e in range(E):
        n = len(idxs[e])
        ygT = res.results[e]["ygT"].reshape(D, C)
        out[idxs[e]] += ygT[:, :n].T
    return out, res


def kernel(**inputs) -> np.ndarray:
    out, _ = _run(inputs)
    return out


# revision 47
# speedup vs baseline: 1.0167x; 1.0136x over previous
"""Trainium2 Bass kernel for an 8-expert top-2 MoE block (T=2048, D=1024, H=4096).

Strategy (expert-parallel, sparse dispatch):
  - Host computes the (tiny) gate: router logits, top-2 selection, softmax
    combine weights.  This *is* the sharding decision: tokens are dispatched
    (gathered) to the core that owns their expert.
  - Core e holds expert e's W1/W2 and runs the FFN only on the tokens routed
    to it (padded to a common capacity C so all 8 cores run one SPMD program).
  - On device: H^T = relu(W1^T X^T + b1), Y^T = (W2^T H^T + b2) * comb, all
    in transposed-activation layout so the contraction dim is always on
    SBUF partitions.
  - Host scatter-adds the per-expert outputs back to the full (T, D) output.
"""

import os
import sys

for p in ("/opt/trn_rl_repo",):
    if p not in sys.path and os.path.isdir(p):
        sys.path.insert(0, p)

# The kernel needs the axon-tunneled NeuronCores; don't let a stray
# JAX_PLATFORMS=cpu (set by some harnesses for the reference) hide them.
if "jax" not in sys.modules and os.environ.get("JAX_PLATFORMS", "") == "cpu":
    del os.environ["JAX_PLATFORMS"]

from contextlib import ExitStack

import numpy as np

import concourse.bass as bass
import concourse.bacc as bacc
import concourse.mybir as mybir
import concourse.tile as tile
from concourse.bass_utils import run_bass_kernel_spmd

T, D, H, E = 2048, 1024, 4096, 8
DC, HC = D // 128, H // 128  # 8, 32 chunks of 128
F32 = mybir.dt.float32
F32R = mybir.dt.float32r  # fp32 bits, single-pass PE mode (4x faster, reduced precision)
BF16 = mybir.dt.bfloat16
FP16 = mybir.dt.float16

# "f32r": fp32 storage, single-pass reduced-precision matmul (~2e-4 rel err)
# "fp16": fp16 operands (~4e-4 rel err), halves weight DMA + faster LDWEIGHTS
# "bf16": bf16 operands (~3e-3 rel err), same speed as fp16
MM_MODE = "fp16"

_prog_cache = {}


def _ntiles(C):
    """Split C (a multiple of 32) into equal-ish chunks of <=512, multiples of 32."""
    nt = -(-C // 512)
    m = C // 32
    sizes = []
    for i in range(nt):
        k = m // nt + (1 if i < m % nt else 0)
        sizes.append(k * 32)
    out, n0 = [], 0
    for s in sizes:
        out.append((n0, s))
        n0 += s
    return out


def _build_program(C, mode):
    """One SPMD program: dense FFN of one expert over C (padded) tokens."""
    MMDT = {"f32r": F32R, "fp16": FP16, "bf16": BF16}[mode]
    nc = bacc.Bacc("TRN2", target_bir_lowering=False, debug=False)

    xg_d = nc.dram_tensor("xg", [128, DC, C], MMDT, kind="ExternalInput")
    w1_d = nc.dram_tensor("w1t", [HC, 128, DC, 128], MMDT, kind="ExternalInput")
    w2_d = nc.dram_tensor("w2t", [DC, 128, HC, 128], MMDT, kind="ExternalInput")
    b1_d = nc.dram_tensor("b1h", [128, HC], F32, kind="ExternalInput")
    b2_d = nc.dram_tensor("b2h", [128, DC], F32, kind="ExternalInput")
    comb_d = nc.dram_tensor("comb", [1, C], F32, kind="ExternalInput")
    out_d = nc.dram_tensor("ygT", [DC, 128, C], F32, kind="ExternalOutput")

    ntiles = _ntiles(C)
    NSZ = ntiles[0][1]

    with tile.TileContext(nc) as tc, ExitStack() as ctx:
        const = ctx.enter_context(tc.tile_pool(name="const", bufs=1))
        w1p = ctx.enter_context(tc.tile_pool(name="w1p", bufs=6))
        w2p = ctx.enter_context(tc.tile_pool(name="w2p", bufs=4))
        hp = ctx.enter_context(tc.tile_pool(name="hp", bufs=1))
        op = ctx.enter_context(tc.tile_pool(name="outp", bufs=4))
        psp = ctx.enter_context(tc.tile_pool(name="ps", bufs=3, space="PSUM"))

        # Head-of-stream: issue w1[0..1] ahead of everything so the first
        # matmul's weights transfer in parallel with the xg chunks.
        w1_head = []
        for hc in range(2):
            w1h = w1p.tile([128, DC, 128], MMDT, name=f"w1h{hc}", tag="w1h")
            nc.sync.dma_start(w1h[:], w1_d[hc])
            w1_head.append(w1h)

        xg = const.tile([128, DC, C], MMDT)
        for dc in range(0, DC, 2):  # split across DMA queues
            nc.sync.dma_start(xg[:, dc : dc + 2, :], xg_d[:, dc : dc + 2, :])
        # small consts on the (idle) gpsimd SWDGE path, off Sync's queue
        b1s = const.tile([128, HC], F32)
        nc.gpsimd.dma_start(b1s[:], b1_d[:])
        b2s = const.tile([128, DC], F32)
        nc.gpsimd.dma_start(b2s[:], b2_d[:])
        combrow = const.tile([1, C], F32)
        nc.gpsimd.dma_start(combrow[:], comb_d[:])
        combb = const.tile([128, C], F32)
        nc.gpsimd.partition_broadcast(combb[:], combrow[:])

        hT = hp.tile([128, HC, C], MMDT)

        # Phase 1: H^T[h, t] = relu(sum_d W1[d, h] * X^T[d, t] + b1[h])
        for hc in range(HC):
            if hc < 2:
                w1h = w1_head[hc]
            else:
                w1h = w1p.tile([128, DC, 128], MMDT, name=f"w1h{hc}", tag="w1h")
                nc.sync.dma_start(w1h[:], w1_d[hc])
            pss = [psp.tile([128, NSZ], F32, name=f"ps{i}", tag=f"ps{i}") for i in range(len(ntiles))]
            for dc in range(DC):
                for ps, (n0, nsz) in zip(pss, ntiles):
                    nc.tensor.matmul(
                        ps[:, :nsz],
                        w1h[:, dc, :],
                        xg[:, dc, n0 : n0 + nsz],
                        start=(dc == 0),
                        stop=(dc == DC - 1),
                    )
            for ps, (n0, nsz) in zip(pss, ntiles):
                nc.scalar.activation(
                    hT[:, hc, n0 : n0 + nsz],
                    ps[:, :nsz],
                    mybir.ActivationFunctionType.Relu,
                    bias=b1s[:, hc : hc + 1],
                )

        # Phase 2: Y^T[d, t] = (sum_h W2[h, d] * H^T[h, t] + b2[d]) * comb[t]
        for dc in range(DC):
            w2d = w2p.tile([128, HC, 128], MMDT)
            for q in range(4):  # split across DMA queues
                nc.sync.dma_start(
                    w2d[:, q * 8 : (q + 1) * 8, :], w2_d[dc, :, q * 8 : (q + 1) * 8, :]
                )
            for i, (n0, nsz) in enumerate(ntiles):
                ps = psp.tile([128, NSZ], F32, name=f"ps{i}", tag=f"ps{i}")
                for hc in range(HC):
                    nc.tensor.matmul(
                        ps[:, :nsz],
                        w2d[:, hc, :],
                        hT[:, hc, n0 : n0 + nsz],
                        start=(hc == 0),
                        stop=(hc == HC - 1),
                    )
                ot = op.tile([128, NSZ], F32)
                nc.vector.scalar_tensor_tensor(
                    ot[:, :nsz],
                    ps[:, :nsz],
                    b2s[:, dc : dc + 1],
                    combb[:, n0 : n0 + nsz],
                    op0=mybir.AluOpType.add,
                    op1=mybir.AluOpType.mult,
                )
                nc.sync.dma_start(out_d[dc, :, n0 : n0 + nsz], ot[:, :nsz])

    nc.compile()
    return nc


def _route(xs, Wg, k):
    """Top-k routing + softmax combine weights, mirroring jax.lax.top_k
    (descending, ties broken by lower index) + softmax over the k logits."""
    router = xs @ Wg.T  # (T, E) fp32
    t = np.arange(xs.shape[0])[:, None]
    sel = np.zeros((xs.shape[0], k), np.int64)
    masked = router.copy()
    for j in range(k):
        sel[:, j] = np.argmax(masked, axis=1)
        masked[t[:, 0], sel[:, j]] = -np.inf
    logits = router[t, sel]  # (T, k), descending
    ex = np.exp((logits - logits[:, :1]).astype(np.float32))
    wgt = (ex / ex.sum(axis=1, keepdims=True)).astype(np.float32)
    return sel, wgt


def _prep_core_inputs(xs, W1e, b1e, W2e, b2e, idx, wgt, C):
    import ml_dtypes

    mmdt = {"f32r": np.float32, "fp16": np.float16, "bf16": ml_dtypes.bfloat16}[
        MM_MODE
    ]
    n = idx.shape[0]
    xsg = np.zeros((C, D), np.float32)
    xsg[:n] = xs[idx]
    xg = np.ascontiguousarray(xsg.T.reshape(DC, 128, C).transpose(1, 0, 2)).astype(mmdt)
    w1t = np.ascontiguousarray(W1e.reshape(DC, 128, HC, 128).transpose(2, 1, 0, 3)).astype(mmdt)
    w2t = np.ascontiguousarray(W2e.reshape(HC, 128, DC, 128).transpose(2, 1, 0, 3)).astype(mmdt)
    b1h = np.ascontiguousarray(b1e.reshape(HC, 128).T)
    b2h = np.ascontiguousarray(b2e.reshape(DC, 128).T)
    comb = np.zeros((1, C), np.float32)
    comb[0, :n] = wgt
    return {"xg": xg, "w1t": w1t, "w2t": w2t, "b1h": b1h, "b2h": b2h, "comb": comb}


def _run(inputs, trace=False, **rk):
    xs = np.asarray(inputs["xs"], np.float32)
    top_k = int(inputs["top_k"])
    Wg = np.asarray(inputs["Wg"], np.float32)
    W1 = np.asarray(inputs["W1"], np.float32)
    b1 = np.asarray(inputs["b1"], np.float32)
    W2 = np.asarray(inputs["W2"], np.float32)
    b2 = np.asarray(inputs["b2"], np.float32)

    sel2, wgt2 = _route(xs, Wg, top_k)
    sel = sel2.ravel()
    wgt = wgt2.ravel()
    tok = np.repeat(np.arange(T), top_k)
    idxs, wgts = [], []
    for e in range(E):
        m = sel == e
        idxs.append(tok[m])
        wgts.append(wgt[m].astype(np.float32))
    C = max(128, -(-max(len(ix) for ix in idxs) // 32) * 32)

    key = (C, MM_MODE)
    if key not in _prog_cache:
        _prog_cache[key] = _build_program(C, MM_MODE)
    nc = _prog_cache[key]

    in_maps = [
        _prep_core_inputs(xs, W1[e], b1[e], W2[e], b2[e], idxs[e], wgts[e], C)
        for e in range(E)
    ]
    res = run_bass_kernel_spmd(nc, in_maps, core_ids=list(range(E)), trace=trace, **rk)

    out = np.zeros((T, D), np.float32)
    for e in range(E):
        n = len(idxs[e])
        ygT = res.results[e]["ygT"].reshape(D, C)
        out[idxs[e]] += ygT[:, :n].T
    return out, res


def kernel(**inputs) -> np.ndarray:
    out, _ = _run(inputs)
    return out


# revision 48
# speedup vs baseline: 1.0192x; 1.0025x over previous
"""Trainium2 Bass kernel for an 8-expert top-2 MoE block (T=2048, D=1024, H=4096).

Strategy (expert-parallel, sparse dispatch):
  - Host computes the (tiny) gate: router logits, top-2 selection, softmax
    combine weights.  This *is* the sharding decision: tokens are dispatched
    (gathered) to the core that owns their expert.
  - Core e holds expert e's W1/W2 and runs the FFN only on the tokens routed
    to it (padded to a common capacity C so all 8 cores run one SPMD program).
  - On device: H^T = relu(W1^T X^T + b1), Y^T = (W2^T H^T + b2) * comb, all
    in transposed-activation layout so the contraction dim is always on
    SBUF partitions.
  - Host scatter-adds the per-expert outputs back to the full (T, D) output.
"""

import os
import sys

for p in ("/opt/trn_rl_repo",):
    if p not in sys.path and os.path.isdir(p):
        sys.path.insert(0, p)

# The kernel needs the axon-tunneled NeuronCores; don't let a stray
# JAX_PLATFORMS=cpu (set by some harnesses for the reference) hide them.
if "jax" not in sys.modules and os.environ.get("JAX_PLATFORMS", "") == "cpu":
    del os.environ["JAX_PLATFORMS"]

from contextlib import ExitStack

import numpy as np

import concourse.bass as bass
import concourse.bacc as bacc
import concourse.mybir as mybir
import concourse.tile as tile
from concourse.bass_utils import run_bass_kernel_spmd

T, D, H, E = 2048, 1024, 4096, 8
DC, HC = D // 128, H // 128  # 8, 32 chunks of 128
F32 = mybir.dt.float32
F32R = mybir.dt.float32r  # fp32 bits, single-pass PE mode (4x faster, reduced precision)
BF16 = mybir.dt.bfloat16
FP16 = mybir.dt.float16

# "f32r": fp32 storage, single-pass reduced-precision matmul (~2e-4 rel err)
# "fp16": fp16 operands (~4e-4 rel err), halves weight DMA + faster LDWEIGHTS
# "bf16": bf16 operands (~3e-3 rel err), same speed as fp16
MM_MODE = "fp16"

_prog_cache = {}


def _ntiles(C):
    """Split C (a multiple of 32) into equal-ish chunks of <=512, multiples of 32."""
    nt = -(-C // 512)
    m = C // 32
    sizes = []
    for i in range(nt):
        k = m // nt + (1 if i < m % nt else 0)
        sizes.append(k * 32)
    out, n0 = [], 0
    for s in sizes:
        out.append((n0, s))
        n0 += s
    return out


def _build_program(C, mode):
    """One SPMD program: dense FFN of one expert over C (padded) tokens."""
    MMDT = {"f32r": F32R, "fp16": FP16, "bf16": BF16}[mode]
    nc = bacc.Bacc("TRN2", target_bir_lowering=False, debug=False)

    xg_d = nc.dram_tensor("xg", [128, DC, C], MMDT, kind="ExternalInput")
    w1_d = nc.dram_tensor("w1t", [HC, 128, DC, 128], MMDT, kind="ExternalInput")
    w2_d = nc.dram_tensor("w2t", [DC, 128, HC, 128], MMDT, kind="ExternalInput")
    b1_d = nc.dram_tensor("b1h", [128, HC], F32, kind="ExternalInput")
    b2_d = nc.dram_tensor("b2h", [128, DC], F32, kind="ExternalInput")
    comb_d = nc.dram_tensor("comb", [1, C], F32, kind="ExternalInput")
    out_d = nc.dram_tensor("ygT", [DC, 128, C], F32, kind="ExternalOutput")

    ntiles = _ntiles(C)
    NSZ = ntiles[0][1]

    with tile.TileContext(nc) as tc, ExitStack() as ctx:
        const = ctx.enter_context(tc.tile_pool(name="const", bufs=1))
        w1p = ctx.enter_context(tc.tile_pool(name="w1p", bufs=8))
        w2p = ctx.enter_context(tc.tile_pool(name="w2p", bufs=6))
        hp = ctx.enter_context(tc.tile_pool(name="hp", bufs=1))
        op = ctx.enter_context(tc.tile_pool(name="outp", bufs=4))
        psp = ctx.enter_context(tc.tile_pool(name="ps", bufs=3, space="PSUM"))

        # Head-of-stream: issue w1[0..1] ahead of everything so the first
        # matmul's weights transfer in parallel with the xg chunks.
        w1_head = []
        for hc in range(2):
            w1h = w1p.tile([128, DC, 128], MMDT, name=f"w1h{hc}", tag="w1h")
            nc.sync.dma_start(w1h[:], w1_d[hc])
            w1_head.append(w1h)

        xg = const.tile([128, DC, C], MMDT)
        for dc in range(0, DC, 2):  # split across DMA queues
            nc.sync.dma_start(xg[:, dc : dc + 2, :], xg_d[:, dc : dc + 2, :])
        # small consts on the (idle) gpsimd SWDGE path, off Sync's queue
        b1s = const.tile([128, HC], F32)
        nc.gpsimd.dma_start(b1s[:], b1_d[:])
        b2s = const.tile([128, DC], F32)
        nc.gpsimd.dma_start(b2s[:], b2_d[:])
        combrow = const.tile([1, C], F32)
        nc.gpsimd.dma_start(combrow[:], comb_d[:])
        combb = const.tile([128, C], F32)
        nc.gpsimd.partition_broadcast(combb[:], combrow[:])

        hT = hp.tile([128, HC, C], MMDT)

        # Phase 1: H^T[h, t] = relu(sum_d W1[d, h] * X^T[d, t] + b1[h])
        for hc in range(HC):
            if hc < 2:
                w1h = w1_head[hc]
            else:
                w1h = w1p.tile([128, DC, 128], MMDT, name=f"w1h{hc}", tag="w1h")
                nc.sync.dma_start(w1h[:], w1_d[hc])
            pss = [psp.tile([128, NSZ], F32, name=f"ps{i}", tag=f"ps{i}") for i in range(len(ntiles))]
            for dc in range(DC):
                for ps, (n0, nsz) in zip(pss, ntiles):
                    nc.tensor.matmul(
                        ps[:, :nsz],
                        w1h[:, dc, :],
                        xg[:, dc, n0 : n0 + nsz],
                        start=(dc == 0),
                        stop=(dc == DC - 1),
                    )
            for ps, (n0, nsz) in zip(pss, ntiles):
                nc.scalar.activation(
                    hT[:, hc, n0 : n0 + nsz],
                    ps[:, :nsz],
                    mybir.ActivationFunctionType.Relu,
                    bias=b1s[:, hc : hc + 1],
                )

        # Phase 2: Y^T[d, t] = (sum_h W2[h, d] * H^T[h, t] + b2[d]) * comb[t]
        for dc in range(DC):
            w2d = w2p.tile([128, HC, 128], MMDT)
            for q in range(4):  # split across DMA queues
                nc.sync.dma_start(
                    w2d[:, q * 8 : (q + 1) * 8, :], w2_d[dc, :, q * 8 : (q + 1) * 8, :]
                )
            for i, (n0, nsz) in enumerate(ntiles):
                ps = psp.tile([128, NSZ], F32, name=f"ps{i}", tag=f"ps{i}")
                for hc in range(HC):
                    nc.tensor.matmul(
                        ps[:, :nsz],
                        w2d[:, hc, :],
                        hT[:, hc, n0 : n0 + nsz],
                        start=(hc == 0),
                        stop=(hc == HC - 1),
                    )
                ot = op.tile([128, NSZ], F32)
                nc.vector.scalar_tensor_tensor(
                    ot[:, :nsz],
                    ps[:, :nsz],
                    b2s[:, dc : dc + 1],
                    combb[:, n0 : n0 + nsz],
                    op0=mybir.AluOpType.add,
                    op1=mybir.AluOpType.mult,
                )
                nc.sync.dma_start(out_d[dc, :, n0 : n0 + nsz], ot[:, :nsz])

    nc.compile()
    return nc


def _route(xs, Wg, k):
    """Top-k routing + softmax combine weights, mirroring jax.lax.top_k
    (descending, ties broken by lower index) + softmax over the k logits."""
    router = xs @ Wg.T  # (T, E) fp32
    t = np.arange(xs.shape[0])[:, None]
    sel = np.zeros((xs.shape[0], k), np.int64)
    masked = router.copy()
    for j in range(k):
        sel[:, j] = np.argmax(masked, axis=1)
        masked[t[:, 0], sel[:, j]] = -np.inf
    logits = router[t, sel]  # (T, k), descending
    ex = np.exp((logits - logits[:, :1]).astype(np.float32))
    wgt = (ex / ex.sum(axis=1, keepdims=True)).astype(np.float32)
    return sel, wgt


def _prep_core_inputs(xs, W1e, b1e, W2e, b2e, idx, wgt, C):
    import ml_dtypes

    mmdt = {"f32r": np.float32, "fp16": np.float16, "bf16": ml_dtypes.bfloat16}[
        MM_MODE
    ]
    n = idx.shape[0]
    xsg = np.zeros((C, D), np.float32)
    xsg[:n] = xs[idx]
    xg = np.ascontiguousarray(xsg.T.reshape(DC, 128, C).transpose(1, 0, 2)).astype(mmdt)
    w1t = np.ascontiguousarray(W1e.reshape(DC, 128, HC, 128).transpose(2, 1, 0, 3)).astype(mmdt)
    w2t = np.ascontiguousarray(W2e.reshape(HC, 128, DC, 128).transpose(2, 1, 0, 3)).astype(mmdt)
    b1h = np.ascontiguousarray(b1e.reshape(HC, 128).T)
    b2h = np.ascontiguousarray(b2e.reshape(DC, 128).T)
    comb = np.zeros((1, C), np.float32)
    comb[0, :n] = wgt
    return {"xg": xg, "w1t": w1t, "w2t": w2t, "b1h": b1h, "b2h": b2h, "comb": comb}


def _run(inputs, trace=False, **rk):
    xs = np.asarray(inputs["xs"], np.float32)
    top_k = int(inputs["top_k"])
    Wg = np.asarray(inputs["Wg"], np.float32)
    W1 = np.asarray(inputs["W1"], np.float32)
    b1 = np.asarray(inputs["b1"], np.float32)
    W2 = np.asarray(inputs["W2"], np.float32)
    b2 = np.asarray(inputs["b2"], np.float32)

    sel2, wgt2 = _route(xs, Wg, top_k)
    sel = sel2.ravel()
    wgt = wgt2.ravel()
    tok = np.repeat(np.arange(T), top_k)
    idxs, wgts = [], []
    for e in range(E):
        m = sel == e
        idxs.append(tok[m])
        wgts.append(wgt[m].astype(np.float32))
    C = max(128, -(-max(len(ix) for ix in idxs) // 32) * 32)

    key = (C, MM_MODE)
    if key not in _prog_cache:
        _prog_cache[key] = _build_program(C, MM_MODE)
    nc = _prog_cache[key]

    in_maps = [
        _prep_core_inputs(xs, W1[e], b1[e], W2[e], b2[e], idxs[e], wgts[e], C)
        for e in range(E)
    ]
    res = run_bass_kernel_spmd(nc, in_maps, core_ids=list(range(E)), trace=trace, **rk)

    out = np.zeros((T, D), np.float32)
    for e in range(E):
        n = len(idxs[e])
        ygT = res.results[e]["ygT"].reshape(D, C)
        out[idxs[e]] += ygT[:, :n].T
    return out, res


def kernel(**inputs) -> np.ndarray:
    out, _ = _run(inputs)
    return out
